# revision 24
# baseline (speedup 1.0000x reference)
"""Trainium2 Bass kernel for modulated conv1d (StyleGAN-style Conv1DMod).

Reference computation (per batch sample b):
  wm[k,c,f]  = kern[k,c,f] * coef * (style[b,c] + 1)        (modulate)
  denom[f]   = rsqrt(sum_{k,c} wm[k,c,f]^2)                 (demodulate)
  out[b,f,w] = denom[f] * sum_{k,c} wm[k,c,f] * feat[b,c,w+k-1]   (SAME conv)

Sharding: data-parallel over batch B=8 -> one sample per NeuronCore.
Demodulation is a per-(b,f) linear scale, so it is applied to the conv
*output* tiles (whose partition dim is f) instead of rescaling weights.

The conv runs as 6 PSUM-accumulated fp32r matmuls (single-pass PE fp32)
per [128f, 512w] output tile; fp32r needs producers to round, so the
feature chunks are DMA'd as fp32 and rounded by the otherwise-idle
Scalar engine.
"""

import numpy as np

import concourse.bass as bass
import concourse.mybir as mybir
import concourse.tile as tile

B, C, W, K, F = 8, 256, 8192, 3, 256
COEF = 1.0 / float(np.sqrt(K * C))

P = 128
CT = C // P  # 2 contraction tiles
FT = F // P  # 2 output-partition tiles
WCHUNK = 2048  # X dma chunk width (1 MB per [128, 2048] f32 transfer)
NJ = W // WCHUNK  # 4 chunks
WTILE = 512  # matmul moving-operand width (fp32 max)
NI = WCHUNK // WTILE  # 4 w-tiles per chunk
XCOLS = WCHUNK + 2  # chunk + 1-col halo each side

MAX_WAITS = 1  # walrus codegen in this container rejects >1 sync wait per inst


def _split_sync_waits(nc, limit=MAX_WAITS):
    """Move excess sem-waits onto NoOps inserted before the offending
    instruction (same engine, program order preserved)."""
    uid = 0
    for fn in nc.m.functions:
        for bb in fn.blocks:
            insts = bb.instructions
            changed = False
            newlist = []
            for ins in insts:
                si = ins.sync_info
                if si is not None and len(si.on_wait) > limit:
                    waits = list(si.on_wait)
                    keep = waits[-limit:]
                    excess = waits[:-limit]
                    for k in range(0, len(excess), limit):
                        nop = mybir.InstNoOp(name=f"waitsplit-{uid}", ins=[], outs=[])
                        uid += 1
                        nop.engine = ins.engine
                        nop.sync_info = mybir.SyncInfo(
                            on_wait=excess[k : k + limit], on_update=[]
                        )
                        newlist.append(nop)
                    ins.sync_info = mybir.SyncInfo(
                        on_wait=keep, on_update=list(si.on_update)
                    )
                    changed = True
                newlist.append(ins)
            if changed:
                bb.instructions = newlist


def _conv1dmod_body(tc, feat, style, kern, out):
    nc = tc.nc
    f32 = mybir.dt.float32
    f32r = mybir.dt.float32r

    with (
        tc.tile_pool(name="xbuf", bufs=1) as xbuf,
        tc.tile_pool(name="xraw", bufs=4) as xraw_pool,
        tc.tile_pool(name="wbuf", bufs=1) as wbuf,
        tc.tile_pool(name="stage", bufs=3) as stage_pool,
        tc.tile_pool(name="psum", bufs=7, space="PSUM") as psum_pool,
        tc.tile_pool(name="dpsum", bufs=1, space="PSUM") as dpsum_pool,
    ):
        # ---- PE warmup: the conv stream starts ~17us in (after the DMA
        # fill), but HAM throttles an idle PE to 1.2 GHz and needs ~3.4us of
        # sustained activity to unthrottle. Burn ~8.5us of dummy matmuls at
        # the head of the PE queue so the real stream starts at full clock.
        scratch = wbuf.tile([P, WTILE], f32, tag="warm")
        nc.vector.memset(scratch[:], 0.3)
        for _ in range(10):
            wp = psum_pool.tile([P, WTILE], f32, tag="psum", name="warm_ps")
            nc.tensor.matmul(wp[:], scratch[:, 0:P], scratch[:], start=True, stop=True)

        # ---- small weight DMAs first: they gate every conv matmul. Keep the
        # SP queue free for the feature chunks: style leads on SP (tiny),
        # kern ct0 pieces ride the Scalar HWDGE queue, ct1 pieces the SWDGE
        # queue. kern [K, C, F] flat is [(2K) x 128, F]: piece a=2k+ct is a
        # fully contiguous 128 KB block landing on partitions c%128.
        ssty = wbuf.tile([P, CT], f32, tag="ssty")
        with nc.allow_non_contiguous_dma(reason="256-elem style vector"):
            nc.sync.dma_start(ssty[:], style.rearrange("(o p) -> p o", p=P))
        kflat = kern.rearrange("k (h p) f -> (k h) p f", p=P)
        ksb = [
            wbuf.tile([P, K, F], f32, tag=f"ksb_{ct}", name=f"ksb_{ct}")
            for ct in range(CT)
        ]
        for k in range(K):
            nc.scalar.dma_start(ksb[0][:, k, :], kflat[2 * k])
            nc.gpsimd.dma_start(ksb[1][:, k, :], kflat[2 * k + 1])

        # ---- modulate weights ----
        s1 = wbuf.tile([P, CT], f32, tag="s1")
        nc.vector.tensor_scalar(
            s1[:], ssty[:], 1.0, COEF, mybir.AluOpType.add, mybir.AluOpType.mult
        )
        wm = []
        for ct in range(CT):
            wmt = wbuf.tile([P, K, F], f32r, tag=f"wm_{ct}")
            nc.vector.tensor_scalar_mul(wmt[:], ksb[ct][:], s1[:, ct : ct + 1])
            wm.append(wmt)

        # ---- feature chunks: DMA fp32, round to fp32r on the Scalar
        # engine (it keeps up: ~2.4us/chunk vs the ~4.8us/chunk DMA feed),
        # leaving the DVE free for the weight chain and demod copies.
        def convert(ct, dst, src):
            nc.scalar.copy(dst, src)

        xt = [[None] * NJ for _ in range(CT)]

        def emit_loads(j, npieces=1):
            for ct in range(CT):
                crow = slice(ct * P, (ct + 1) * P)
                t = xbuf.tile([P, XCOLS], f32r, tag=f"x_{ct}_{j}")
                xt[ct][j] = t
                lo = j * WCHUNK - 1
                hi = j * WCHUNK + WCHUNK + 1
                dst_lo = 0
                if lo < 0:
                    nc.vector.memset(t.bitcast(f32)[:, 0:1], 0.0)
                    dst_lo = 1
                    lo = 0
                if hi > W:
                    nc.vector.memset(t.bitcast(f32)[:, XCOLS - 1 : XCOLS], 0.0)
                    hi = W
                bounds = np.linspace(lo, hi, npieces + 1).astype(int)
                for p0, p1 in zip(bounds[:-1], bounds[1:]):
                    ncols = int(p1 - p0)
                    off = dst_lo + int(p0 - lo)
                    raw = xraw_pool.tile([P, XCOLS], f32, tag="xraw")
                    nc.sync.dma_start(raw[:, off : off + ncols], feat[crow, p0:p1])
                    convert(ct, t[:, off : off + ncols], raw[:, off : off + ncols])

        def emit_mms(j, ft):
            """Emit the NI psum accumulation groups for (chunk j, ft)."""
            pss = []
            for i in range(NI):
                ps = psum_pool.tile([P, WTILE], f32, tag="psum")
                first = True
                for ct in range(CT):
                    for k in range(K):
                        nc.tensor.matmul(
                            ps[:],
                            wm[ct][:, k, ft * P : (ft + 1) * P],
                            xt[ct][j][:, i * WTILE + k : i * WTILE + k + WTILE],
                            start=first,
                            stop=(ct == CT - 1 and k == K - 1),
                        )
                        first = False
                pss.append(ps)
            return pss

        def emit_copies(j, ft, pss):
            """Demodulating PSUM->SBUF copies + half-chunk output stores."""
            st = stage_pool.tile([P, WCHUNK], f32, tag="stage")
            for i, ps in enumerate(pss):
                nc.vector.tensor_scalar_mul(
                    st[:, i * WTILE : (i + 1) * WTILE], ps[:], denom[:, ft : ft + 1]
                )
            out_rows = slice(ft * P, (ft + 1) * P)
            # finer stores on the last chunk shorten the end-of-kernel tail
            npieces = 4 if j == NJ - 1 else 2
            piece = WCHUNK // npieces
            for h in range(npieces):
                out_cols = slice(j * WCHUNK + h * piece, j * WCHUNK + (h + 1) * piece)
                nc.sync.dma_start(
                    out[out_rows, out_cols], st[:, h * piece : (h + 1) * piece]
                )

        # chunk-0 loads + its first matmul block go ahead of everything else
        emit_loads(0, npieces=2)
        pss00 = emit_mms(0, 0)

        # ---- demodulation scale: denom[f] = rsqrt(sum_{k,c} wm^2) ----
        # Emitted after the first conv block so the tiny demod matmuls do
        # not sit at the head of the in-order PE queue waiting on the DVE
        # square/sum chain.
        ssq = []
        for ct in range(CT):
            sqt = wbuf.tile([P, K, F], f32, tag=f"sq_{ct}")
            nc.vector.tensor_mul(sqt[:], wm[ct].bitcast(f32)[:], wm[ct].bitcast(f32)[:])
            sst = wbuf.tile([P, F], f32, tag=f"ssq_{ct}")
            nc.vector.tensor_add(sst[:], sqt[:, 0], sqt[:, 1])
            nc.vector.tensor_add(sst[:], sst[:], sqt[:, 2])
            ssq.append(sst)
        ones = wbuf.tile([P, 1], f32, tag="ones")
        nc.vector.memset(ones[:], 1.0)
        dp = dpsum_pool.tile([P, FT], f32, tag="dpsum")
        for ft in range(FT):
            for ct in range(CT):
                nc.tensor.matmul(
                    dp[:, ft : ft + 1],
                    ssq[ct][:, ft * P : (ft + 1) * P],
                    ones[:],
                    start=(ct == 0),
                    stop=(ct == CT - 1),
                )
        denom = wbuf.tile([P, FT], f32, tag="denom")
        nc.scalar.activation(denom[:], dp[:], mybir.ActivationFunctionType.Sqrt)
        nc.vector.reciprocal(denom[:], denom[:])

        # ---- conv: chunk loads stay one chunk ahead of the matmul stream ----
        emit_loads(1)
        emit_copies(0, 0, pss00)
        emit_copies(0, 1, emit_mms(0, 1))
        for j in range(1, NJ):
            if j + 1 < NJ:
                emit_loads(j + 1)
            for ft in range(FT):
                emit_copies(j, ft, emit_mms(j, ft))


def build_bass():
    nc = bass.Bass(name="conv1dmod")
    feat = nc.dram_tensor("feature", [C, W], mybir.dt.float32, kind="ExternalInput")
    style = nc.dram_tensor("style", [C], mybir.dt.float32, kind="ExternalInput")
    kern = nc.dram_tensor("kern", [K, C, F], mybir.dt.float32, kind="ExternalInput")
    out = nc.dram_tensor("out", [F, W], mybir.dt.float32, kind="ExternalOutput")
    with tile.TileContext(nc) as tc:
        _conv1dmod_body(tc, feat, style, kern, out)
    _split_sync_waits(nc)
    return nc


_NC_CACHE = None


def kernel(feature, style, kernel):
    """Full-input entry point: shard over batch across 8 cores, run, gather."""
    global _NC_CACHE
    from concourse.bass_utils import run_bass_kernel_spmd

    if _NC_CACHE is None:
        _NC_CACHE = build_bass()
    nc = _NC_CACHE

    feature = np.ascontiguousarray(feature, dtype=np.float32)
    style = np.ascontiguousarray(style, dtype=np.float32)
    kernel = np.ascontiguousarray(kernel, dtype=np.float32)

    in_maps = [
        {"feature": feature[b], "style": style[b], "kern": kernel} for b in range(B)
    ]
    res = run_bass_kernel_spmd(nc, in_maps, core_ids=list(range(B)))
    return np.stack([r["out"] for r in res.results], axis=0)
